# revision 19
# baseline (speedup 1.0000x reference)
"""Pairwise max-margin hinge loss on 8 trn2 cores.

reference:  o = out.T [B=32, n=2048]; l = label.T
            hinge[b,i,j] = max(margin - (o_i-o_j)(l_i-l_j), 0); loss = sum/(2n)

Math used here:
  (o_i-o_j)(l_i-l_j) = p_i + p_j - o_i*l_j - l_i*o_j   with p = o*l
  -> rank-4: M_b = A_b^T @ R_b with A rows [p, 1, -o, -l], R rows [1, p, l, o]
  M_b (and the hinge matrix) is symmetric, so with 128x128 blocks:
     sum(full) = 2*sum(strict-upper off-diag blocks) + sum(diagonal blocks)
  exactly (diag blocks are themselves symmetric and contain the diagonal once).

Per core: 4 batch rows. PE generates blocks into PSUM via K=4 float32r
matmuls; ACT does relu(margin-x)+row-accum, DVE does min(x,margin)-margin
(= -hinge) + row-accum (tensor_scalar accum reduces with op1=add).
Final: column totals via ones^T @ acc matmuls, weighted (+-2/+-1)/(2n) with
activation-copy accums, reduced to one scalar per core; host sums 8 scalars.
"""

import sys

sys.path.insert(0, "/opt/trn_rl_repo")

import numpy as np

N = 2048
B = 32
NCORES = 8
BPC = B // NCORES  # 4 batches per core
NI = N // 128  # 16 i-chunks
NJ = N // 512  # 4 j-groups
DIV = float(N * 2)  # 4096

_cache = {}
DEBUG_DUMP = False


def _build(margin: float):
    import concourse.bacc as bacc
    import concourse.bass as bass
    import concourse.tile as tile
    from concourse import mybir

    f32 = mybir.dt.float32
    bf16 = mybir.dt.bfloat16
    Alu = mybir.AluOpType
    Act = mybir.ActivationFunctionType

    # ---- host-side job planning -------------------------------------------
    # matmul job: (q, i0, col0, width, subjobs); subjob: (sub_off, sub_w, weight)
    mm_jobs = []
    for q in range(BPC):
        for ci in range(NI):
            cj0, r = divmod(ci, 4)
            i0 = 128 * ci
            # merged diag(+rect) tile: cols [512*cj0 + 128*r, 512*(cj0+1))
            w = 512 - 128 * r
            subs = [(0, 128, 1)]
            if w > 128:
                subs.append((128, w - 128, 2))
            mm_jobs.append((q, i0, 512 * cj0 + 128 * r, w, subs))
            for cj in range(cj0 + 1, NJ):
                mm_jobs.append((q, i0, 512 * cj, 512, [(0, 512, 2)]))

    # greedy engine assignment for consumer subjobs, balancing engine time
    t_act = 0.0
    t_dve = 0.0
    assign = {}  # (job_idx, sub_idx) -> ("act"|"dve")
    act_cols = {1: [], 2: []}  # weight -> list of subjob keys
    dve_cols = {1: [], 2: []}
    for ji, (q, i0, c0, w, subs) in enumerate(mm_jobs):
        for si, (so, sw, wt) in enumerate(subs):
            if t_act + sw / 1.2 <= t_dve + sw / 0.96:
                assign[(ji, si)] = "act"
                t_act += sw / 1.2
                act_cols[wt].append((ji, si))
            else:
                assign[(ji, si)] = "dve"
                t_dve += sw / 0.96
                dve_cols[wt].append((ji, si))
    na2, na1 = len(act_cols[2]), len(act_cols[1])
    nd2, nd1 = len(dve_cols[2]), len(dve_cols[1])
    na, nd = na2 + na1, nd2 + nd1
    col_of = {}
    for idx, key in enumerate(act_cols[2] + act_cols[1]):
        col_of[key] = idx
    for idx, key in enumerate(dve_cols[2] + dve_cols[1]):
        col_of[key] = idx
    # DVE accumulates sum(min(x, margin)); hinge sum per col = w*margin - acc.
    # The w*margin part is a compile-time constant, added host-side per core.
    width_of = {
        (ji, si): subs[si][1]
        for ji, (q, i0, c0, w, subs) in enumerate(mm_jobs)
        for si in range(len(subs))
    }
    dve_w2 = sum(width_of[k] for k in dve_cols[2])
    dve_w1 = sum(width_of[k] for k in dve_cols[1])
    host_const = margin * (2.0 * dve_w2 + 1.0 * dve_w1) * 128.0 / DIV

    # ---- build program -----------------------------------------------------
    nc = bacc.Bacc(
        "TRN2",
        target_bir_lowering=False,
        debug=False,
        enable_asserts=False,
        num_devices=NCORES,
    )
    # bf16 hi/lo split: x = Ah.T@Rh + Ah.T@Rl + Al.T@Rh as one K=12 bf16
    # matmul (rows [Ah;Ah;Al] x [Rh;Rl;Rh]); ~fp32 accuracy at bf16 speed.
    A_d = nc.dram_tensor("A", [12, BPC * N], bf16, kind="ExternalInput").ap()
    R_d = nc.dram_tensor("R", [12, BPC * N], bf16, kind="ExternalInput").ap()
    res_d = nc.dram_tensor("res", [1, 1], f32, kind="ExternalOutput").ap()
    if DEBUG_DUMP:
        dbg_a_d = nc.dram_tensor("dbg_a", [128, max(na, 1)], f32, kind="ExternalOutput").ap()
        dbg_d_d = nc.dram_tensor("dbg_d", [128, max(nd, 1)], f32, kind="ExternalOutput").ap()
        dbg_f_d = nc.dram_tensor("dbg_f", [1, 4], f32, kind="ExternalOutput").ap()

    with tile.TileContext(nc) as tc:
        with (
            tc.tile_pool(name="io", bufs=1) as io,
            tc.tile_pool(name="psum", bufs=6, space="PSUM") as psum,
            tc.tile_pool(name="psfin", bufs=1, space="PSUM") as psfin,
            tc.tile_pool(name="dumpa", bufs=2) as dumpa,
            tc.tile_pool(name="dumpd", bufs=2) as dumpd,
        ):
            A = io.tile([12, BPC * N], bf16, tag="A")
            R = io.tile([12, BPC * N], bf16, tag="R")
            nc.sync.dma_start(out=A[:], in_=A_d[:])
            nc.sync.dma_start(out=R[:], in_=R_d[:])

            acc_a = io.tile([128, max(na, 1)], f32, tag="acca")
            acc_d = io.tile([128, max(nd, 1)], f32, tag="accd")
            ones = io.tile([128, 1], f32, tag="ones")
            nc.gpsimd.memset(ones[:], 1.0)

            for ji, (q, i0, c0, w, subs) in enumerate(mm_jobs):
                pt = psum.tile([128, 512], f32, tag="pt")
                nc.tensor.matmul(
                    pt[:, :w],
                    A[0:12, q * N + i0 : q * N + i0 + 128],
                    R[0:12, q * N + c0 : q * N + c0 + w],
                    start=True,
                    stop=True,
                )
                for si, (so, sw, wt) in enumerate(subs):
                    col = col_of[(ji, si)]
                    if assign[(ji, si)] == "act":
                        dmp = dumpa.tile([128, 512], f32, tag="da")
                        nc.scalar.activation(
                            dmp[:, :sw],
                            pt[:, so : so + sw],
                            Act.Relu,
                            bias=margin,
                            scale=-1.0,
                            accum_out=acc_a[:, col : col + 1],
                        )
                    else:
                        dmp = dumpd.tile([128, 512], f32, tag="dd")
                        nc.vector.tensor_scalar(
                            dmp[:, :sw],
                            pt[:, so : so + sw],
                            margin,
                            0.0,
                            Alu.min,
                            Alu.add,
                            accum_out=acc_d[:, col : col + 1],
                        )

            # ---- final reduction to one scalar ----------------------------
            psA = psfin.tile([1, max(na, 1)], f32, tag="psA")
            psD = psfin.tile([1, max(nd, 1)], f32, tag="psD")
            nc.tensor.matmul(psA[:], ones[:], acc_a[:], start=True, stop=True)
            nc.tensor.matmul(psD[:], ones[:], acc_d[:], start=True, stop=True)

            fin4 = io.tile([1, 4], f32, tag="fin4")
            nc.gpsimd.memset(fin4[:], 0.0)
            dmp = dumpa.tile([128, 512], f32, tag="da")
            groups = [
                (psA, 0, na2, 2.0 / DIV, 0),
                (psA, na2, na1, 1.0 / DIV, 1),
                (psD, 0, nd2, -2.0 / DIV, 2),
                (psD, nd2, nd1, -1.0 / DIV, 3),
            ]
            for src, off, cnt, scale, k in groups:
                if cnt == 0:
                    continue
                nc.scalar.activation(
                    dmp[0:1, :cnt],
                    src[0:1, off : off + cnt],
                    Act.Copy,
                    scale=scale,
                    accum_out=fin4[0:1, k : k + 1],
                )
            res_sb = io.tile([1, 1], f32, tag="res")
            nc.vector.tensor_reduce(
                res_sb[0:1, 0:1], fin4[0:1, 0:4], mybir.AxisListType.X, Alu.add
            )
            nc.sync.dma_start(out=res_d[:], in_=res_sb[:])
            if DEBUG_DUMP:
                nc.sync.dma_start(out=dbg_a_d[:], in_=acc_a[:])
                nc.sync.dma_start(out=dbg_d_d[:], in_=acc_d[:])
                nc.sync.dma_start(out=dbg_f_d[:], in_=fin4[:])

    nc.compile()
    return nc, host_const


def _get(margin: float):
    key = margin
    if key not in _cache:
        _cache[key] = _build(margin)
    return _cache[key]


def _in_maps(out, label):
    import ml_dtypes

    bf = ml_dtypes.bfloat16
    ot = np.ascontiguousarray(out.T).astype(np.float32, copy=False)  # [32, 2048]
    lt = np.ascontiguousarray(label.T).astype(np.float32, copy=False)
    p = ot * lt
    maps = []
    for c in range(NCORES):
        A = np.empty((4, BPC * N), np.float32)
        R = np.empty((4, BPC * N), np.float32)
        for q in range(BPC):
            b = BPC * c + q
            s = slice(q * N, (q + 1) * N)
            A[0, s] = p[b]
            A[1, s] = 1.0
            A[2, s] = -ot[b]
            A[3, s] = -lt[b]
            R[0, s] = 1.0
            R[1, s] = p[b]
            R[2, s] = lt[b]
            R[3, s] = ot[b]
        Ah = A.astype(bf)
        Al = (A - Ah.astype(np.float32)).astype(bf)
        Rh = R.astype(bf)
        Rl = (R - Rh.astype(np.float32)).astype(bf)
        A12 = np.concatenate([Ah, Ah, Al], axis=0)  # [12, BPC*N]
        R12 = np.concatenate([Rh, Rl, Rh], axis=0)
        maps.append({"A": A12, "R": R12})
    return maps


def kernel(out, label, margin):
    from concourse.bass_utils import run_bass_kernel_spmd

    margin_f = float(np.asarray(margin))
    nc, host_const = _get(margin_f)
    maps = _in_maps(np.asarray(out), np.asarray(label))
    br = run_bass_kernel_spmd(nc, maps, list(range(NCORES)))
    total = NCORES * host_const
    for r in br.results:
        total += float(np.asarray(r["res"]).reshape(-1)[0])
    return np.float32(total)
